# revision 11
# baseline (speedup 1.0000x reference)
"""DDSC transformer kernel for Trainium2, 8 NeuronCores, batch-parallel.

Model (per sample): x[6,512] -> pw init_enc -> 6 transformer blocks
(DDSC q/k/v projections = pointwise conv + softmax-gated multi-kernel
depthwise conv; per-head attention over 512 positions; instance norms;
FFN) -> sigmoid head -> probs[512].

Distribution: pure data parallel. B=32 samples, 8 cores, 4 samples/core.
On-chip layout: canonical state tile [128, 512] = (4 samples x 32
channels) on partitions, sequence on free dim.

Key tricks:
- DDSC = dense 1D conv with folded kernel W2[c,i,j] = pw[c,i]*gate_comb[c,j]
  (15 taps); computed on the PE via a 4-shifted im2col (K=480 dense).
- q/k scaled by C^-0.25 folded into weights.
- dot[k,q] computed with k-positions on PSUM partitions so the softmax
  denominator is a ones-vector matmul; exp has no max-subtraction (|dot|
  < 0.02 for this model's weight scale -- verified against reference).
- attention normalization folded into the PSUM->SBUF evacuation multiply.
- matmuls in float32r (full-rate fp32 PE mode); 32x32 tile_position
  packing for the C=32-contraction matmuls.
- instance-norm rstd via exp(-0.5*ln(var+eps)) to stay in one ACT table
  set (natural_log_exp) for the whole body; one switch to sigmoid at end.
"""

import os
import sys

import numpy as np

for _p in ("/opt/trn_rl_repo", "/root/.axon_site/_ro/trn_rl_repo"):
    if os.path.isdir(_p) and _p not in sys.path:
        sys.path.insert(0, _p)

import concourse.bass as bass
import concourse.tile as tile
from concourse import bacc, mybir
from concourse.bass_utils import run_bass_kernel_spmd

F32 = mybir.dt.float32
F32R = mybir.dt.float32r
BF16 = mybir.dt.bfloat16
AF = mybir.ActivationFunctionType
ALU = mybir.AluOpType

N_CORES = 8
B, IN_CH, L = 32, 6, 512
C, H, DM, DEPTH = 32, 8, 4, 6
KS = (3, 15)
EPS = 1e-5
BS = B // N_CORES  # samples per core = 4
CH = C * H  # 256
QK_SCALE = float(C) ** -0.25

# weight-tile column offsets ([128, WCOLS] per block)
QOFF, KOFF, VOFF = 0, 1024, 2048
UOFF, F1OFF, F2OFF, NOFF = 3072, 3136, 3264, 3296
WCOLS = 3300


# --------------------------------------------------------------------------
# host-side weight folding
# --------------------------------------------------------------------------

def _softmax(v):
    e = np.exp(v - v.max())
    return e / e.sum()


def _fold_ddsc(pp, extra_scale=1.0):
    """Fold pw + gated depthwise kernels into lhsT layout [128, 4, 256].

    W2[c,i,j] = pw[c,i] * wc[c,j], wc = g1*dw15 + g0*dw3 (center taps).
    lhsT[g][32*dj + i, c] = W2[c, i, 4*g + dj]  (zero beyond tap 14).
    """
    pw = np.asarray(pp["pw"], np.float64)          # [256, 32]
    g = _softmax(np.asarray(pp["gate"], np.float64))
    dw3 = np.asarray(pp["dw"][0], np.float64)[:, 0, :]   # [256, 3]
    dw15 = np.asarray(pp["dw"][1], np.float64)[:, 0, :]  # [256, 15]
    wc = g[1] * dw15
    wc[:, 6:9] += g[0] * dw3
    W2 = pw[:, :, None] * wc[:, None, :] * extra_scale   # [256, 32, 15]
    lhsT = np.zeros((128, 4, CH), np.float32)
    for gg in range(4):
        for dj in range(4):
            j = 4 * gg + dj
            if j < 15:
                lhsT[32 * dj:32 * dj + 32, gg, :] = W2[:, :, j].T
    return lhsT


def _prep_weights(params):
    """Build wblk [DEPTH, 128, WCOLS] and winit [128, 33] numpy arrays."""
    wblk = np.zeros((DEPTH, 128, WCOLS), np.float32)
    for bi, bp in enumerate(params["blocks"]):
        wb = wblk[bi]
        wb[:, QOFF:QOFF + 1024] = _fold_ddsc(bp["q"], QK_SCALE).reshape(128, 1024)
        wb[:, KOFF:KOFF + 1024] = _fold_ddsc(bp["k"], QK_SCALE).reshape(128, 1024)
        wb[:, VOFF:VOFF + 1024] = _fold_ddsc(bp["v"], 1.0).reshape(128, 1024)
        uni = np.asarray(bp["unify"], np.float32)        # [32, 256]
        for ch in range(2):
            # UT_ch[hc, o] = unify[o, 128*ch + hc]
            wb[:, UOFF + 32 * ch:UOFF + 32 * ch + 32] = uni[:, 128 * ch:128 * ch + 128].T
        f1 = np.asarray(bp["ff1"], np.float32)           # [128, 32]
        f2 = np.asarray(bp["ff2"], np.float32)           # [32, 128]
        for s in range(BS):
            wb[32 * s:32 * s + 32, F1OFF:F1OFF + 128] = f1.T
            wb[32 * s:32 * s + 32, NOFF + 0] = np.asarray(bp["n1g"], np.float32)
            wb[32 * s:32 * s + 32, NOFF + 1] = np.asarray(bp["n1b"], np.float32)
            wb[32 * s:32 * s + 32, NOFF + 2] = np.asarray(bp["n2g"], np.float32)
            wb[32 * s:32 * s + 32, NOFF + 3] = np.asarray(bp["n2b"], np.float32)
        wb[:, F2OFF:F2OFF + 32] = f2.T  # [128, 32]
    # bf16 weights for column-tiled matmuls: unify UT (2x32) + ff2T (32)
    wblk16 = np.zeros((DEPTH, 128, 96), np.float32)
    for bi, bp in enumerate(params["blocks"]):
        uni = np.asarray(bp["unify"], np.float32)
        for ch in range(2):
            wblk16[bi, :, 32 * ch:32 * ch + 32] = uni[:, 128 * ch:128 * ch + 128].T
        wblk16[bi, :, 64:96] = np.asarray(bp["ff2"], np.float32).T
    import ml_dtypes
    wblk16 = wblk16.astype(ml_dtypes.bfloat16)
    # winit [128, 132]: cols 0:128 = block-diag init lhsT (rows 0-23);
    # cols 128:132 = block-diag final-head lhsT
    winit = np.zeros((128, 148), np.float32)
    ie = np.asarray(params["init_enc"], np.float32)      # [32, 6]
    wo = np.asarray(params["out"], np.float32)           # [1, 32]
    for s in range(BS):
        for i in range(IN_CH):
            winit[6 * s + i, 32 * s:32 * s + 32] = ie[:, i]
        winit[32 * s:32 * s + 32, 128 + s] = wo[0]
    return wblk, wblk16, winit


# --------------------------------------------------------------------------
# device kernel
# --------------------------------------------------------------------------

def _r(ap):
    return ap if ap.dtype == F32R else ap.bitcast(F32R)


def _spread4(ap_128xN):
    """AP view selecting partitions {0, 32, 64, 96} of a [128, N] tile."""
    return ap_128xN.rearrange("(g r) l -> g r l", g=4)[:, 0:1, :]


def _inorm(nc, pools, Y, gcol, bcol, epsap):
    """InstanceNorm over free dim per partition row; returns new state tile."""
    smalls, state = pools["smalls"], pools["state"]
    st = smalls.tile([128, 6], F32, tag="st", name="st")
    nc.vector.bn_stats(out=st, in_=Y)
    mv = smalls.tile([128, 2], F32, tag="mv", name="mv")
    nc.vector.bn_aggr(out=mv, in_=st)
    lnv = smalls.tile([128, 1], F32, tag="lnv", name="lnv")
    nc.scalar.activation(out=lnv, in_=mv[:, 1:2], func=AF.Ln,
                         bias=epsap[:, 0:1], scale=1.0)
    rstd = smalls.tile([128, 1], F32, tag="rstd", name="rstd")
    nc.scalar.activation(out=rstd, in_=lnv, func=AF.Exp, scale=-0.5)
    sc = smalls.tile([128, 1], F32, tag="sc", name="sc")
    nc.vector.tensor_mul(out=sc, in0=rstd, in1=gcol)
    bi = smalls.tile([128, 1], F32, tag="bi", name="bi")
    nc.vector.tensor_mul(out=bi, in0=mv[:, 0:1], in1=sc)
    bi2 = smalls.tile([128, 1], F32, tag="bi2", name="bi2")
    nc.vector.tensor_sub(out=bi2, in0=bcol, in1=bi)
    Xn = state.tile([128, 512], F32R, tag="state", name="Xn")
    nc.vector.tensor_scalar(out=Xn, in0=Y, scalar1=sc, scalar2=bi2,
                            op0=ALU.mult, op1=ALU.add)
    return Xn


def _block(nc, pools, X, wb, wb16, epsap, ones, zeros16, blk):
    state = pools["state"]
    x4p, qkp, vtp, attnp, aop = (pools[k] for k in ("x4p", "qkp", "vtp", "attnp", "aop"))
    pdot, pacc, pu = pools["pdot"], pools["pacc"], pools["pu"]
    ffp, smalls = pools["ffp"], pools["smalls"]

    pu_t = pu.tile([128, 512], F32, tag="pu", name=f"pu{blk}")
    ao_chunks_by_s = []
    for s in range(BS):
        # ---- im2col: x4[32*dj + i, t] = x_pad[i, t + dj], halo 7 ----
        x4 = x4p.tile([128, 528], F32R, tag="x4", name=f"x4_{blk}_{s}")
        nc.vector.tensor_copy(out=x4[:, 0:8], in_=zeros16[:, 0:8])
        nc.vector.tensor_copy(out=x4[:, 512:528], in_=zeros16[:, 0:16])
        for dj in range(4):
            nc.sync.dma_start(out=x4[32 * dj:32 * dj + 32, 7 - dj:519 - dj],
                              in_=X[32 * s:32 * s + 32, :])
        # ---- q/k projections: out[c,l], c on partitions (2 chunks) ----
        q_sb = qkp.tile([128, 2, 512], F32R, tag="q", name=f"q_{blk}_{s}")
        k_sb = qkp.tile([128, 2, 512], F32R, tag="k", name=f"k_{blk}_{s}")
        for dst, off in ((q_sb, QOFF), (k_sb, KOFF)):
            for mc in range(2):
                pq = pacc.tile([128, 512], F32, tag="pacc", name="pq")
                for g in range(4):
                    lo = off + g * 256 + mc * 128
                    nc.tensor.matmul(pq, _r(wb[:, lo:lo + 128]),
                                     _r(x4[:, 4 * g:4 * g + 512]),
                                     start=(g == 0), stop=(g == 3),
                                     tile_position=(0, 0))
                nc.vector.tensor_copy(out=dst[:, mc, :], in_=pq)
        # ---- v projection, transposed: vt[l, c] (4 l-chunks) ----
        vt = vtp.tile([128, 4, 256], BF16, tag="vt", name=f"vt_{blk}_{s}")
        for lc in range(4):
            pv = pacc.tile([128, 512], F32, tag="pacc", name="pv")
            for g in range(4):
                lo = 4 * g + 128 * lc
                nc.tensor.matmul(pv[:, 0:256], _r(x4[:, lo:lo + 128]),
                                 _r(wb[:, VOFF + g * 256:VOFF + g * 256 + 256]),
                                 start=(g == 0), stop=(g == 3),
                                 tile_position=(0, 0))
            nc.vector.tensor_copy(out=vt[:, lc, :], in_=pv[:, 0:256])
        # ---- attention ----
        ao_chunks = []
        for quad in range(2):  # heads 4*quad .. 4*quad+3
            attn_tiles = {}
            for pi in range(2):
                heads = (4 * quad + 2 * pi, 4 * quad + 2 * pi + 1)
                for m in range(4):
                    pd = pdot.tile([128, 2, 512], F32, tag="pdot", name="pd")
                    for hh, h in enumerate(heads):
                        rg = 32 * (h % 4)
                        nc.tensor.matmul(
                            pd[:, hh, :],
                            _r(k_sb[rg:rg + 32, h // 4, 128 * m:128 * m + 128]),
                            _r(q_sb[rg:rg + 32, h // 4, :]),
                            start=True, stop=True, tile_position=(rg, 0))
                    at = attnp.tile([128, 2, 512], BF16, tag="attn", name="at")
                    nc.scalar.activation(out=at, in_=pd, func=AF.Exp)
                    attn_tiles[(pi, m)] = at
            # A@V + denominators for the 4 heads of this quad
            ao_ps = pacc.tile([128, 512], F32, tag="pacc", name="ao_ps")
            sm_ps = pacc.tile([128, 512], F32, tag="pacc", name="sm_ps")
            for m in range(4):
                for pi in range(2):
                    for hh in range(2):
                        h = 4 * quad + 2 * pi + hh
                        cg = 32 * (h % 4)
                        at = attn_tiles[(pi, m)]
                        nc.tensor.matmul(
                            ao_ps[cg:cg + 32, :],
                            vt[:, m, 32 * h:32 * h + 32], at[:, hh, :],
                            start=(m == 0), stop=(m == 3),
                            tile_position=(0, cg))
                for pi in range(2):
                    for hh in range(2):
                        h = 4 * quad + 2 * pi + hh
                        cg = 32 * (h % 4)
                        at = attn_tiles[(pi, m)]
                        nc.tensor.matmul(
                            sm_ps[cg:cg + 1, :],
                            ones[:, 0:1], at[:, hh, :],
                            start=(m == 0), stop=(m == 3),
                            tile_position=(0, cg))
            # normalize: ao = ao_ps * (1/sums) broadcast down each 32-row group
            rc = aop.tile([128, 512], F32, tag="rc", name="rc")
            # full-tile recip: only rows {0,32,64,96} (the per-head sums) are
            # ever read downstream; other lanes compute unread garbage.
            nc.vector.reciprocal_approx_fast(out=rc, in_=sm_ps)
            rs = aop.tile([128, 512], F32, tag="rs", name="rs")
            nc.vector.stream_shuffle(out=rs, in_=rc, mask=[0] * 32)
            ao_sb = aop.tile([128, 512], BF16, tag="ao", name="ao_sb")
            nc.vector.tensor_mul(out=ao_sb, in0=ao_ps, in1=rs)
            ao_chunks.append(ao_sb)
        ao_chunks_by_s.append(ao_chunks)
        # ---- unify into shared per-block psum (rows 32*s) ----
        for ch in range(2):
            nc.tensor.matmul(pu_t[32 * s:32 * s + 32, :],
                             wb16[:, 32 * ch:32 * ch + 32],
                             ao_chunks[ch],
                             start=(ch == 0), stop=(ch == 1),
                             tile_position=(0, 32 * s))
    # ---- residual + inorm1 ----
    Y = state.tile([128, 512], F32, tag="state", name=f"Y{blk}")
    nc.vector.tensor_add(out=Y, in0=pu_t, in1=X)
    X1 = _inorm(nc, pools, Y, wb[:, NOFF + 0:NOFF + 1], wb[:, NOFF + 1:NOFF + 2], epsap)
    # ---- FFN ----
    hts = []
    for half in range(2):
        phs = []
        for j in range(2):
            s = 2 * half + j
            rg = 32 * s
            ph = pacc.tile([128, 512], F32, tag="pacc", name="ph")
            nc.tensor.matmul(ph, _r(wb[rg:rg + 32, F1OFF:F1OFF + 128]),
                             _r(X1[rg:rg + 32, :]),
                             start=True, stop=True, tile_position=(rg, 0))
            phs.append(ph)
        for j in range(2):
            ht = ffp.tile([128, 512], BF16, tag="ff", name="ht")
            nc.vector.tensor_scalar_max(out=ht, in0=phs[j], scalar1=0.0)
            hts.append(ht)
    pf = pacc.tile([128, 512], F32, tag="pacc", name="pf")
    for s in range(BS):
        nc.tensor.matmul(pf[32 * s:32 * s + 32, :],
                         wb16[:, 64:96], hts[s],
                         start=True, stop=True, tile_position=(0, 32 * s))
    # ---- residual + inorm2 ----
    Y2 = state.tile([128, 512], F32, tag="state", name=f"Y2{blk}")
    nc.vector.tensor_add(out=Y2, in0=pf, in1=X1)
    X2 = _inorm(nc, pools, Y2, wb[:, NOFF + 2:NOFF + 3], wb[:, NOFF + 3:NOFF + 4], epsap)
    return X2


def _emit(tc, out_d, xin_d, wblk_d, wblk16_d, winit_d, dbg_d=None):
    nc = tc.nc
    from contextlib import ExitStack
    ctx = ExitStack()
    pools = {}
    for name, bufs, space in (
        ("wpool", 2, "SBUF"), ("singles", 1, "SBUF"), ("state", 4, "SBUF"),
        ("x4p", 2, "SBUF"), ("qkp", 4, "SBUF"), ("vtp", 2, "SBUF"),
        ("attnp", 10, "SBUF"), ("aop", 6, "SBUF"), ("ffp", 5, "SBUF"),
        ("smalls", 8, "SBUF"),
        ("pdot", 2, "PSUM"), ("pacc", 3, "PSUM"), ("pu", 1, "PSUM"),
    ):
        pools[name] = ctx.enter_context(tc.tile_pool(name=name, bufs=bufs, space=space))
    singles, state, pacc = pools["singles"], pools["state"], pools["pacc"]

    ones = singles.tile([128, 1], BF16, name="ones")
    nc.vector.memset(ones, 1.0)
    epsap = singles.tile([128, 1], F32, name="epsap")
    nc.vector.memset(epsap, EPS)
    wini = singles.tile([128, 148], F32R, name="wini")
    nc.sync.dma_start(out=wini, in_=winit_d.bitcast(F32R))
    zeros16 = wini[:, 132:148]
    xin_sb = singles.tile([128, 512], F32R, name="xin_sb")
    nc.sync.dma_start(out=xin_sb[0:BS * IN_CH, :], in_=xin_d.bitcast(F32R))

    # init encoder: 4 diagonal 32x32 tiles, K=6
    p0 = pacc.tile([128, 512], F32, tag="pacc", name="p0")
    nc.tensor.matmul(p0, wini[0:4 * IN_CH, 0:128], xin_sb[0:4 * IN_CH, :],
                     start=True, stop=True, tile_position=(0, 0))
    X = state.tile([128, 512], F32R, tag="state", name="X0")
    nc.vector.tensor_copy(out=X, in_=p0)
    if dbg_d is not None:
        nc.sync.dma_start(out=dbg_d[0], in_=X.bitcast(F32))

    for blk in range(DEPTH):
        wb = pools["wpool"].tile([128, WCOLS], F32R, tag="wb", name=f"wb{blk}")
        nc.sync.dma_start(out=wb, in_=wblk_d[blk].bitcast(F32R))
        wb16 = pools["wpool"].tile([128, 96], BF16, tag="wb16", name=f"wb16_{blk}")
        nc.sync.dma_start(out=wb16, in_=wblk16_d[blk])
        X = _block(nc, pools, X, wb, wb16, epsap, ones, zeros16, blk)
        if dbg_d is not None:
            nc.sync.dma_start(out=dbg_d[blk + 1], in_=X.bitcast(F32))

    # output head: probs[s, l] = sigmoid(w_out . X[s])
    pfin = pacc.tile([128, 512], F32, tag="pacc", name="pfin")
    nc.tensor.matmul(pfin[0:4, :], wini[:, 128:132], X,
                     start=True, stop=True, tile_position=(0, 0))
    probs = pools["aop"].tile([128, 512], F32, tag="rc", name="probs")
    nc.scalar.activation(out=probs[0:4, :], in_=pfin[0:4, :], func=AF.Sigmoid)
    nc.sync.dma_start(out=out_d, in_=probs[0:4, :])
    ctx.close()


_PROGRAM = {}


def _build_program(debug=False):
    if debug in _PROGRAM:
        return _PROGRAM[debug]
    nc = bacc.Bacc("TRN2", target_bir_lowering=False, debug=False)
    xin_d = nc.dram_tensor("xin", [BS * IN_CH, L], F32, kind="ExternalInput").ap()
    wblk_d = nc.dram_tensor("wblk", [DEPTH, 128, WCOLS], F32, kind="ExternalInput").ap()
    wblk16_d = nc.dram_tensor("wblk16", [DEPTH, 128, 96], BF16, kind="ExternalInput").ap()
    winit_d = nc.dram_tensor("winit", [128, 148], F32, kind="ExternalInput").ap()
    out_d = nc.dram_tensor("out", [BS, L], F32, kind="ExternalOutput").ap()
    dbg_d = None
    if debug:
        dbg_d = nc.dram_tensor("dbg", [DEPTH + 1, 128, 512], F32,
                               kind="ExternalOutput").ap()
    with tile.TileContext(nc) as tc:
        _emit(tc, out_d, xin_d, wblk_d, wblk16_d, winit_d, dbg_d)
    nc.compile()
    _PROGRAM[debug] = nc
    return nc


# --------------------------------------------------------------------------
# entry point
# --------------------------------------------------------------------------

def kernel(x, params):
    x = np.ascontiguousarray(np.asarray(x, np.float32))
    assert x.shape == (B, IN_CH, L), x.shape
    wblk, wblk16, winit = _prep_weights(params)
    nc = _build_program()
    in_maps = []
    for c in range(N_CORES):
        in_maps.append({
            "xin": np.ascontiguousarray(x[c * BS:(c + 1) * BS].reshape(BS * IN_CH, L)),
            "wblk": wblk,
            "wblk16": wblk16,
            "winit": winit,
        })
    res = run_bass_kernel_spmd(nc, in_maps, list(range(N_CORES)))
    out = np.empty((B, L), np.float32)
    for c in range(N_CORES):
        out[c * BS:(c + 1) * BS] = res.results[c]["out"]
    return out
